# revision 12
# baseline (speedup 1.0000x reference)
# Cost-volume concatenation kernel for Trainium2 (Bass/Tile), SPMD over 8 cores.
#
# Problem: left, right: [B=2, H=64, W=256, C=32] f32.
# out[b, d+48, h, w, :32] = left[b,h,w,:]  * valid(w,d)
# out[b, d+48, h, w, 32:] = right[b,h,w-d,:] * valid(w,d),  d in [-48, 48)
# valid(w,d) = 0 <= w-d < W.  Output [2, 96, 64, 256, 64] f32 (~805 MB).
#
# Sharding: W axis (not disparity). Core k owns output columns
# w in [32k, 32k+32) for ALL 96 disparity levels. Rationale: the kernel is
# write-bound (each core writes ~100.7 MB regardless of sharding), so the
# only free variable is input read traffic, which shares the same ~358 GB/s
# per-NC HBM budget. Disparity sharding needs all of left+right per core
# (~8.8 MB); W-sharding needs only the core's 32 left columns (0.5 MB) plus
# a 128-column window of right (2.1 MB) covering all +-48 shifts — ~2.7 MB
# total, saving ~17 us of HBM time per core.
#
# The kernel program is identical on every core; per-core variation lives in
# the DATA:
#   - lslice: left[:, :, 32k:32k+32, :]                      [128p, 32w*32c]
#   - rpad:   right columns [32k-48, 32k+80) zero-padded     [128p, 128t*32c]
#             where out of [0, W); rpad[p, t] = right col 32k-48+t.
#             The zero padding implements right-half validity masking.
#   - vrep:   0/1 validity mask vrep[p, t] = (0 <= 32k-48+t < W), replicated
#             across partitions; out_left = lslice * vrep_shifted implements
#             the left-half masking.
#
# SBUF layout: partitions = (h, b) h-major — p = 2*h + b, 128 partitions;
# free dim = (w, c). The per-core output DRAM buffer is laid out
# [D2, H, B, WSH, 2C] (disparity outermost) so each disparity level's
# 1 MB write is ONE fully contiguous DRAM region (128 descriptors x 8 KB,
# consecutive addresses). DRAM write locality matters: a layout whose
# per-DMA descriptors scatter across the 100 MB buffer measures ~22 GB/s
# per SDMA engine vs ~27 GB/s for compact footprints. The host-side
# transpose back to [B, D2, H, w, 2C] is absorbed by the np.concatenate
# copy it already does.
#
# Per disparity j (d = j-48) the shifted right window starts at t0 = 96-j,
# so all shifts are in [1, 96] and slices stay inside the 128-col window.
# Per-core traffic: ~2.7 MB read + ~100.7 MB write (write-roofline bound).

import numpy as np

B, H, W, C = 2, 64, 256, 32
MAX_DISP = 48
D2 = 2 * MAX_DISP            # 96 disparity levels (all on every core)
N_CORES = 8
WSH = W // N_CORES           # 32 output columns per core
TPAD = 128                   # right window: cols [32k-48, 32k+80), 128 wide
P = B * H                    # 128 SBUF partitions = (h, b) h-major
F32 = np.float32

SPLIT_T = 48                 # rpad head: t < 48 (first j levels read t < 44)

_CACHE = {}


def _build_nc():
    import concourse.bacc as bacc
    import concourse.mybir as mybir
    from concourse.tile import TileContext, add_dep_helper

    f32 = mybir.dt.float32
    nc = bacc.Bacc("TRN2", target_bir_lowering=False, debug=False)
    left_t = nc.dram_tensor("lslice", [P, WSH * C], f32, kind="ExternalInput")
    rpad_t = nc.dram_tensor("rpad", [P, TPAD * C], f32, kind="ExternalInput")
    vrep_t = nc.dram_tensor("vrep", [P, TPAD], f32, kind="ExternalInput")
    # [D2, (h b w c)]: one contiguous 1 MB row per disparity level.
    LVL = H * B * WSH * 2 * C  # 262144 f32 = 1 MB per level
    out_t = nc.dram_tensor("out", [D2, LVL], f32, kind="ExternalOutput")
    out_ap = out_t.ap()

    with TileContext(nc) as tc:
        with (
            tc.tile_pool(name="ins", bufs=1) as ipool,
            tc.tile_pool(name="outs", bufs=8) as opool,
        ):
            left_sb = ipool.tile([P, WSH * C], f32, tag="lslice")
            rpad_sb = ipool.tile([P, TPAD * C], f32, tag="rpad")
            vrep_sb = ipool.tile([P, TPAD], f32, tag="vrep")
            # Phased input loads: the head (~1.4 MB) drains alone at full read
            # bandwidth so the first output DMA can start a few us in; the
            # rpad tail drains underneath the first output DMAs.
            head = [
                nc.sync.dma_start(out=vrep_sb[:], in_=vrep_t[:]),
                nc.sync.dma_start(out=left_sb[:], in_=left_t[:]),
                nc.sync.dma_start(
                    out=rpad_sb[:, : SPLIT_T * C], in_=rpad_t[:, : SPLIT_T * C]
                ),
            ]
            tail = [
                nc.scalar.dma_start(
                    out=rpad_sb[:, SPLIT_T * C :], in_=rpad_t[:, SPLIT_T * C :]
                ),
            ]
            for t_ in tail:
                for h_ in head:
                    add_dep_helper(
                        t_.ins, h_.ins,
                        reason="input tail loads drain after head loads",
                    )

            lv = left_sb[:].rearrange("p (w c) -> p w c", c=C)
            rv = rpad_sb[:].rearrange("p (t c) -> p t c", c=C)
            vv = vrep_sb[:]  # [p, t]; broadcast across c inside the mul

            # j descending => shifted windows t0 = 96-j ascend, so the head
            # load (t < SPLIT_T) covers the first levels.
            for j in reversed(range(D2)):
                ot = opool.tile([P, WSH * 2 * C], f32, tag="ot")
                ov = ot[:].rearrange("p (w c) -> p w c", c=2 * C)
                t0 = D2 - j  # in [1, 96]
                nc.vector.tensor_mul(
                    out=ov[:, :, 0:C],
                    in0=lv[:, :, :],
                    in1=vv[:, t0 : t0 + WSH, None].broadcast_to([P, WSH, C]),
                )
                nc.vector.tensor_copy(
                    out=ov[:, :, C : 2 * C],
                    in_=rv[:, t0 : t0 + WSH, :],
                )
                nc.sync.dma_start(out=out_ap[j : j + 1, :], in_=ot[:])
    nc.finalize()
    return nc


def get_nc():
    if "nc" not in _CACHE:
        _CACHE["nc"] = _build_nc()
    return _CACHE["nc"]


def _hb_major(x):
    """[B, H, rest...] -> [128 = (h, b) h-major, prod(rest)] contiguous."""
    return np.ascontiguousarray(x.transpose(1, 0, 2, 3)).reshape(P, -1)


def prep_inputs(left, right):
    """Build the 8 per-core input maps from full left/right."""
    left = np.ascontiguousarray(left, dtype=F32)
    right = np.ascontiguousarray(right, dtype=F32)
    in_maps = []
    for k in range(N_CORES):
        base = WSH * k - MAX_DISP  # rpad[..., t, :] = right[..., base + t, :]
        lslice = _hb_major(left[:, :, WSH * k : WSH * (k + 1), :])
        rpad = np.zeros((B, H, TPAD, C), F32)
        lo, hi = max(0, -base), min(TPAD, W - base)
        rpad[:, :, lo:hi, :] = right[:, :, lo + base : hi + base, :]
        vk = np.zeros(TPAD, F32)
        vk[lo:hi] = 1.0
        vrep = np.ascontiguousarray(np.broadcast_to(vk, (P, TPAD)))
        in_maps.append({"lslice": lslice, "rpad": _hb_major(rpad), "vrep": vrep})
    return in_maps


def run(left, right, **kwargs):
    """Run the SPMD kernel; returns (full_output, BassKernelResults)."""
    from concourse.bass_utils import run_bass_kernel_spmd

    nc = get_nc()
    in_maps = prep_inputs(left, right)
    try:
        res = run_bass_kernel_spmd(
            nc, in_maps, core_ids=list(range(N_CORES)), **kwargs
        )
    except Exception:
        # The axon/neuron device occasionally reports a transient
        # NRT_EXEC_UNIT_UNRECOVERABLE on a cold first run; a retry succeeds.
        res = run_bass_kernel_spmd(
            nc, in_maps, core_ids=list(range(N_CORES)), **kwargs
        )
    full = np.concatenate(
        [
            r["out"].reshape(D2, H, B, WSH, 2 * C).transpose(2, 0, 1, 3, 4)
            for r in res.results
        ],
        axis=3,
    )
    return full, res


def kernel(left, right):
    full, _ = run(left, right)
    return full


# revision 19
# speedup vs baseline: 1.1688x; 1.1688x over previous
# Cost-volume concatenation kernel for Trainium2 (Bass/Tile), SPMD over 8 cores.
#
# Problem: left, right: [B=2, H=64, W=256, C=32] f32.
# out[b, d+48, h, w, :32] = left[b,h,w,:]  * valid(w,d)
# out[b, d+48, h, w, 32:] = right[b,h,w-d,:] * valid(w,d),  d in [-48, 48)
# valid(w,d) = 0 <= w-d < W.  Output [2, 96, 64, 256, 64] f32 (~805 MB).
#
# Sharding: W axis (not disparity). Core k owns output columns
# w in [32k, 32k+32) for ALL 96 disparity levels. Rationale: the kernel is
# write-bound (each core writes ~100.7 MB regardless of sharding), so the
# only free variable is input read traffic, which shares the same ~358 GB/s
# per-NC HBM budget. Disparity sharding needs all of left+right per core
# (~8.8 MB); W-sharding needs only the core's 32 left columns (0.5 MB) plus
# a 128-column window of right (2.1 MB) covering all +-48 shifts — ~2.7 MB
# total, saving ~17 us of HBM time per core.
#
# The kernel program is identical on every core; per-core variation lives in
# the DATA:
#   - lslice: left[:, :, 32k:32k+32, :]                      [128p, 32w*32c]
#   - rpad:   right columns [32k-48, 32k+80) zero-padded     [128p, 128t*32c]
#             where out of [0, W); rpad[p, t] = right col 32k-48+t.
#             The zero padding implements right-half validity masking.
#   - vrep:   0/1 validity mask vrep[p, t] = (0 <= 32k-48+t < W), replicated
#             across partitions; out_left = lslice * vrep_shifted implements
#             the left-half masking.
#
# SBUF layout: partitions = (h, b) h-major — p = 2*h + b, 128 partitions;
# free dim = (j_within, w, c). The per-core output DRAM buffer is laid out
# [blk=24, (h b) 128, j_within=4, (w c) 2048] — j-blocks of 4 levels,
# partition-major within a block — so each block's 4 MB DMA is ONE fully
# contiguous DRAM region AND each partition contributes a single 32 KB
# contiguous descriptor (128 descriptors/DMA, 8 per SDMA engine). Both
# properties measured to matter: scattered descriptor footprints run
# ~22 GB/s per engine vs ~27 GB/s compact, and 1 MB DMAs (8 KB
# descriptors) leave engines idle between FIFO handoffs on the HWDGE ring
# (concurrency 13.4/16). The host-side unpack back to [B, D2, H, w, 2C]
# is absorbed by the np.concatenate copy it already does.
#
# Per disparity j (d = j-48) the shifted right window starts at t0 = 96-j,
# so all shifts are in [1, 96] and slices stay inside the 128-col window.
# Per-core traffic: ~2.7 MB read + ~100.7 MB write (write-roofline bound).

import numpy as np

B, H, W, C = 2, 64, 256, 32
MAX_DISP = 48
D2 = 2 * MAX_DISP            # 96 disparity levels (all on every core)
N_CORES = 8
WSH = W // N_CORES           # 32 output columns per core
TPAD = 128                   # right window: cols [32k-48, 32k+80), 128 wide
P = B * H                    # 128 SBUF partitions = (h, b) h-major
F32 = np.float32

J_BLK = 4                    # disparity levels per output tile / 4 MB DMA
N_BLK = D2 // J_BLK          # 24 j-blocks
SPLIT_T = 48                 # rpad head: t < 48 (first 4 j-blocks covered)

_CACHE = {}


def _build_nc():
    import concourse.bacc as bacc
    import concourse.mybir as mybir
    from concourse.tile import TileContext, add_dep_helper

    f32 = mybir.dt.float32
    nc = bacc.Bacc("TRN2", target_bir_lowering=False, debug=False)
    left_t = nc.dram_tensor("lslice", [P, WSH * C], f32, kind="ExternalInput")
    rpad_t = nc.dram_tensor("rpad", [P, TPAD * C], f32, kind="ExternalInput")
    vrep_t = nc.dram_tensor("vrep", [P, TPAD], f32, kind="ExternalInput")
    # [N_BLK, (p j w c)]: one contiguous 4 MB row per j-block.
    BLK = P * J_BLK * WSH * 2 * C  # 1048576 f32 = 4 MB per block
    out_t = nc.dram_tensor("out", [N_BLK, BLK], f32, kind="ExternalOutput")
    out_ap = out_t.ap()

    with TileContext(nc) as tc:
        with (
            tc.tile_pool(name="ins", bufs=1) as ipool,
            tc.tile_pool(name="outs", bufs=4) as opool,
        ):
            left_sb = ipool.tile([P, WSH * C], f32, tag="lslice")
            rpad_sb = ipool.tile([P, TPAD * C], f32, tag="rpad")
            vrep_sb = ipool.tile([P, TPAD], f32, tag="vrep")
            # Phased input loads: the head (~1.4 MB) drains alone at full read
            # bandwidth so the first output DMA can start a few us in; the
            # rpad tail drains underneath the first output DMAs.
            head = [
                nc.sync.dma_start(out=vrep_sb[:], in_=vrep_t[:]),
                nc.sync.dma_start(out=left_sb[:], in_=left_t[:]),
                nc.sync.dma_start(
                    out=rpad_sb[:, : SPLIT_T * C], in_=rpad_t[:, : SPLIT_T * C]
                ),
            ]
            tail = [
                nc.scalar.dma_start(
                    out=rpad_sb[:, SPLIT_T * C :], in_=rpad_t[:, SPLIT_T * C :]
                ),
            ]
            for t_ in tail:
                for h_ in head:
                    add_dep_helper(
                        t_.ins, h_.ins,
                        reason="input tail loads drain after head loads",
                    )

            lv = left_sb[:].rearrange("p (w c) -> p w c", c=C)
            rv = rpad_sb[:].rearrange("p (t c) -> p t c", c=C)
            vv = vrep_sb[:]  # [p, t]; broadcast across c inside the mul

            # blocks descending => shifted windows t0 = 96-j ascend, so the
            # head load (t < SPLIT_T) covers the first blocks.
            for blk in reversed(range(N_BLK)):
                ot = opool.tile([P, J_BLK * WSH * 2 * C], f32, tag="ot")
                ov = ot[:].rearrange("p (j w c) -> p j w c", j=J_BLK, c=2 * C)
                for jj in range(J_BLK):
                    t0 = D2 - (blk * J_BLK + jj)  # in [1, 96]
                    nc.vector.tensor_mul(
                        out=ov[:, jj, :, 0:C],
                        in0=lv[:, :, :],
                        in1=vv[:, t0 : t0 + WSH, None].broadcast_to([P, WSH, C]),
                    )
                    nc.vector.tensor_copy(
                        out=ov[:, jj, :, C : 2 * C],
                        in_=rv[:, t0 : t0 + WSH, :],
                    )
                nc.sync.dma_start(out=out_ap[blk : blk + 1, :], in_=ot[:])
    nc.finalize()
    return nc


def get_nc():
    if "nc" not in _CACHE:
        _CACHE["nc"] = _build_nc()
    return _CACHE["nc"]


def _hb_major(x):
    """[B, H, rest...] -> [128 = (h, b) h-major, prod(rest)] contiguous."""
    return np.ascontiguousarray(x.transpose(1, 0, 2, 3)).reshape(P, -1)


def prep_inputs(left, right):
    """Build the 8 per-core input maps from full left/right."""
    left = np.ascontiguousarray(left, dtype=F32)
    right = np.ascontiguousarray(right, dtype=F32)
    in_maps = []
    for k in range(N_CORES):
        base = WSH * k - MAX_DISP  # rpad[..., t, :] = right[..., base + t, :]
        lslice = _hb_major(left[:, :, WSH * k : WSH * (k + 1), :])
        rpad = np.zeros((B, H, TPAD, C), F32)
        lo, hi = max(0, -base), min(TPAD, W - base)
        rpad[:, :, lo:hi, :] = right[:, :, lo + base : hi + base, :]
        vk = np.zeros(TPAD, F32)
        vk[lo:hi] = 1.0
        vrep = np.ascontiguousarray(np.broadcast_to(vk, (P, TPAD)))
        in_maps.append({"lslice": lslice, "rpad": _hb_major(rpad), "vrep": vrep})
    return in_maps


def run(left, right, **kwargs):
    """Run the SPMD kernel; returns (full_output, BassKernelResults)."""
    from concourse.bass_utils import run_bass_kernel_spmd

    nc = get_nc()
    in_maps = prep_inputs(left, right)
    try:
        res = run_bass_kernel_spmd(
            nc, in_maps, core_ids=list(range(N_CORES)), **kwargs
        )
    except Exception:
        # The axon/neuron device occasionally reports a transient
        # NRT_EXEC_UNIT_UNRECOVERABLE on a cold first run; a retry succeeds.
        res = run_bass_kernel_spmd(
            nc, in_maps, core_ids=list(range(N_CORES)), **kwargs
        )
    # out row blk = [(h b) p, j_within, w, 2C]; j = blk * J_BLK + jj.
    full = np.empty((B, D2, H, W, 2 * C), F32)
    for k, r in enumerate(res.results):
        full[:, :, :, WSH * k : WSH * (k + 1), :] = (
            r["out"]
            .reshape(N_BLK, H, B, J_BLK, WSH, 2 * C)
            .transpose(2, 0, 3, 1, 4, 5)
            .reshape(B, D2, H, WSH, 2 * C)
        )
    return full, res


def kernel(left, right):
    full, _ = run(left, right)
    return full
